# revision 5
# baseline (speedup 1.0000x reference)
"""Trainium2 Bass kernel for the ContextComputer GNN message-passing module.

Computation (per batch row b):
    W1, W2 = W[:D], W[D:]
    u_j    = memory_j * mask_j                       # [N, D]
    a_i    = memory_i @ W1 + bias                    # [N, D]
    c_j    = mask_j * (memory_j @ W2)                # [N, D]
    ctx_i  = sum_{j != i} sigmoid(a_i + c_j) * u_j

Sharding: pure data parallel over batch B across the 8 NeuronCores
(B=8192 -> 1024 rows per core); W/b replicated.

Per-core layout: batch rows on the 128 SBUF partitions, features on the
free axis. v2 design notes:
  - memory is cast-loaded fp32->bf16 straight into SBUF (SWDGE cast DMA,
    no DRAM scratch), and the d-on-partition tiles for the matmuls come
    from ONE SBUF->SBUF xbar transpose per row-tile ([128, 3072] ->
    24 planes of [128d, 128rows]).
  - a'_i = m_i @ W1 + 1*bias accumulates in PSUM (bias via a rank-1 ones
    matmul); c_j = mask_j * (m_j @ W2) applies the mask as a per-partition
    scale in the ScalarE PSUM->SBUF copy.
  - Pairwise stage: one wide DVE/ACT instruction per i-pair over all 6 j
    (diagonal included - cheaper than splitting the instruction).
  - The off-diagonal 5-term j-sum runs on the Tensor engine: 4 identity
    matmuls accumulate p_ij (SBUF bf16) into PSUM fp32, and one DVE op
    fuses the 5th term with the PSUM read, writing the fp32 output tile.
    This moves the old 3-op DVE reduction tree onto PE (which has slack)
    and improves accumulation precision (fp32 instead of bf16).
  - Emission is software-pipelined one row-tile ahead (produce stage of
    tile k+1 before pairwise stage of tile k) so ScalarE's PSUM->SBUF
    copies fill the gap while DVE computes the adds.
"""

import numpy as np

import concourse.bass as bass
import concourse.mybir as mybir
import concourse.tile as tile
from concourse.bass_utils import run_bass_kernel_spmd

B, N, D = 8192, 6, 512
P = 128
DC = D // P  # 4 contraction chunks of 128
NCORES = 8
BLOC = B // NCORES

F32 = mybir.dt.float32
BF16 = mybir.dt.bfloat16

_ADD = mybir.AluOpType.add
_MULT = mybir.AluOpType.mult
_IS_EQUAL = mybir.AluOpType.is_equal
_SIGMOID = mybir.ActivationFunctionType.Sigmoid

_nc_cache = {}


def _split_excess_waits(nc, max_waits=1):
    """The pinned walrus build only supports one sync-wait slot per
    instruction; hoist extra Tile-emitted waits onto standalone
    same-engine EventSemaphore instructions (NX dispatcher-level waits,
    so ordering semantics are preserved)."""
    f = nc.m.functions[0]
    for blk in f.blocks:
        new = []
        for ins in blk.instructions:
            si = getattr(ins, "sync_info", None)
            eng = getattr(ins, "engine", None)
            if si is not None and si.on_wait and len(si.on_wait) > max_waits and eng is not None:
                waits = list(si.on_wait)
                extra, keep = waits[:-max_waits], waits[-max_waits:]
                for k, w in enumerate(extra):
                    new.append(
                        mybir.InstEventSemaphore(
                            name=f"{ins.name}_xw{k}",
                            opcode="EventSemaphore",
                            engine=eng,
                            ins=[],
                            outs=[],
                            sync_info=mybir.SyncInfo(on_wait=[w], on_update=[]),
                        )
                    )
                si.on_wait = keep
            new.append(ins)
        blk.instructions[:] = new


def build(bloc=BLOC, split_waits=True):
    nbt = bloc // P
    nc = bass.Bass(num_swdge_queues=4)
    mem = nc.declare_dram_parameter("memory", [bloc, N, D], F32, isOutput=False)
    msk = nc.declare_dram_parameter("mask", [bloc, N, 1], F32, isOutput=False)
    w_p = nc.declare_dram_parameter("W", [2 * D, D], F32, isOutput=False)
    b_p = nc.declare_dram_parameter("b", [D], F32, isOutput=False)
    out = nc.declare_dram_parameter("context", [bloc, N, D], F32, isOutput=True)

    with tile.TileContext(nc) as tc:
        with (
            tc.tile_pool(name="const", bufs=1) as constp,
            tc.tile_pool(name="m", bufs=3) as mp,
            tc.tile_pool(name="mt", bufs=3) as mtp,
            tc.tile_pool(name="maskp", bufs=4) as maskp,
            tc.tile_pool(name="up", bufs=3) as upool,
            tc.tile_pool(name="ap", bufs=3) as apool,
            tc.tile_pool(name="cp", bufs=3) as cpool,
            tc.tile_pool(name="pair", bufs=2) as pairp,
            tc.tile_pool(name="outp", bufs=8) as outp,
            tc.tile_pool(name="psum", bufs=2, space="PSUM") as psp,
            tc.tile_pool(name="ctxps", bufs=4, space="PSUM") as ctxps,
        ):
            # ---- constants: W (cast to bf16), bias, ones row, identity ----
            wt = {}
            for h in range(2):  # 0 -> W1, 1 -> W2
                for dc in range(DC):
                    t = constp.tile([P, D], BF16, tag=f"w{h}{dc}")
                    nc.gpsimd.dma_start(
                        out=t[:], in_=w_p[h * D + dc * P : h * D + (dc + 1) * P, :]
                    )
                    wt[h, dc] = t
            bias_t = constp.tile([1, D], BF16, tag="bias")
            nc.gpsimd.dma_start(out=bias_t[:], in_=b_p[None, :])
            ones_t = constp.tile([1, P], BF16, tag="ones")
            nc.vector.memset(ones_t[:], 1.0)
            ident = constp.tile([P, P], BF16, tag="ident")
            nc.vector.memset(ident[:], 1.0)
            nc.gpsimd.affine_select(
                out=ident[:],
                in_=ident[:],
                pattern=[[-1, P]],
                base=0,
                channel_multiplier=1,
                compare_op=_IS_EQUAL,
                fill=0.0,
            )

            state = {}

            def produce(bt):
                """loads + transpose + u + matmuls + PSUM->SBUF copies"""
                bsl = slice(bt * P, (bt + 1) * P)
                m_t = mp.tile([P, N * D], BF16, tag="m")
                nc.gpsimd.dma_start(
                    out=m_t.rearrange("p (n d) -> p n d", n=N), in_=mem[bsl]
                )
                mask_t = maskp.tile([P, N], F32, tag="mask")
                nc.sync.dma_start(out=mask_t[:], in_=msk[bsl, :, 0])
                # one SBUF->SBUF xbar transpose: planes [n*DC+dc] = [128d, 128rows]
                mt_t = mtp.tile([P, N * DC, P], BF16, tag="mt")
                nc.sync.dma_start(out=mt_t[:], in_=m_t[:], transpose=True)
                # u_j = mask_j * m_j (bf16, 4x tensor_scalar)
                u_all = upool.tile([P, N * D], BF16, tag="u")
                for j in range(N):
                    nc.vector.tensor_scalar_mul(
                        out=u_all[:, j * D : (j + 1) * D],
                        in0=m_t[:, j * D : (j + 1) * D],
                        scalar1=mask_t[:, j : j + 1],
                    )
                a_all = apool.tile([P, N * D], BF16, tag="a")
                c_all = cpool.tile([P, N * D], BF16, tag="c")
                for i in range(N):
                    a_ps = psp.tile([P, D], F32, tag="aps")
                    for dc in range(DC):
                        nc.tensor.matmul(
                            out=a_ps[:],
                            lhsT=mt_t[:, i * DC + dc, :],
                            rhs=wt[0, dc][:],
                            start=(dc == 0),
                            stop=False,
                        )
                    nc.tensor.matmul(
                        out=a_ps[:],
                        lhsT=ones_t[:],
                        rhs=bias_t[:],
                        start=False,
                        stop=True,
                    )
                    nc.scalar.copy(out=a_all[:, i * D : (i + 1) * D], in_=a_ps[:])
                for j in range(N):
                    c_ps = psp.tile([P, D], F32, tag="cps")
                    for dc in range(DC):
                        nc.tensor.matmul(
                            out=c_ps[:],
                            lhsT=mt_t[:, j * DC + dc, :],
                            rhs=wt[1, dc][:],
                            start=(dc == 0),
                            stop=(dc == DC - 1),
                        )
                    # c_j = mask_j * (m_j @ W2): scale in the PSUM->SBUF copy
                    nc.scalar.mul(
                        out=c_all[:, j * D : (j + 1) * D],
                        in_=c_ps[:],
                        mul=mask_t[:, j : j + 1],
                    )
                state[bt] = (u_all, a_all, c_all)

            def pairwise(bt):
                """adds + sigmoid + mult + PE j-sum + output DMA"""
                bsl = slice(bt * P, (bt + 1) * P)
                u_all, a_all, c_all = state.pop(bt)
                for i0 in range(0, N, 2):
                    # t[(i,j)] = a_i + c_j for i in {i0, i0+1}, all j
                    a_b = (
                        a_all[:, i0 * D : (i0 + 2) * D]
                        .rearrange("p (i f) -> p i f", i=2)
                        .rearrange("p i (j f) -> p i j f", j=1)
                        .broadcast_to([P, 2, N, D])
                    )
                    c_b = (
                        c_all.rearrange("p (i f) -> p i f", i=1)
                        .broadcast_to([P, 2, N * D])
                        .rearrange("p i (j f) -> p i j f", j=N)
                    )
                    t_all = pairp.tile([P, 2 * N * D], BF16, tag="t")
                    nc.vector.tensor_tensor(
                        out=t_all.rearrange("p (i j f) -> p i j f", i=2, j=N),
                        in0=a_b,
                        in1=c_b,
                        op=_ADD,
                    )
                    g_all = pairp.tile([P, 2 * N * D], BF16, tag="g")
                    nc.scalar.activation(out=g_all[:], in_=t_all[:], func=_SIGMOID)
                    u_b = (
                        u_all.rearrange("p (i f) -> p i f", i=1)
                        .broadcast_to([P, 2, N * D])
                    )
                    p_all = pairp.tile([P, 2 * N * D], BF16, tag="p")
                    nc.vector.tensor_tensor(
                        out=p_all.rearrange("p (i f) -> p i f", i=2),
                        in0=g_all.rearrange("p (i f) -> p i f", i=2),
                        in1=u_b,
                        op=_MULT,
                    )
                    for il in range(2):
                        i = i0 + il
                        js = [j for j in range(N) if j != i]
                        s_ps = ctxps.tile([P, D], F32, tag="s")
                        for k, j in enumerate(js[:-1]):
                            nc.tensor.matmul(
                                out=s_ps[:],
                                lhsT=ident[:],
                                rhs=p_all[:, (il * N + j) * D : (il * N + j + 1) * D],
                                start=(k == 0),
                                stop=(k == len(js) - 2),
                            )
                        jl = js[-1]
                        ctx_t = outp.tile([P, D], F32, tag="ctx")
                        nc.vector.tensor_add(
                            out=ctx_t[:],
                            in0=s_ps[:],
                            in1=p_all[:, (il * N + jl) * D : (il * N + jl + 1) * D],
                        )
                        nc.scalar.dma_start(out=out[bsl, i, :], in_=ctx_t[:])

            prev = None
            for bt in range(nbt):
                produce(bt)
                if prev is not None:
                    pairwise(prev)
                prev = bt
            pairwise(prev)
    if split_waits:
        _split_excess_waits(nc)
    return nc


def get_nc(bloc=BLOC):
    if bloc not in _nc_cache:
        _nc_cache[bloc] = build(bloc)
    return _nc_cache[bloc]


last_results = None


def kernel(**inputs):
    global last_results
    memory = np.ascontiguousarray(inputs["memory"], dtype=np.float32)
    mask = np.ascontiguousarray(inputs["mask"], dtype=np.float32)
    W = np.ascontiguousarray(inputs["W"], dtype=np.float32)
    b = np.ascontiguousarray(inputs["b"], dtype=np.float32)

    nc = get_nc()
    in_maps = [
        {
            "memory": memory[c * BLOC : (c + 1) * BLOC],
            "mask": mask[c * BLOC : (c + 1) * BLOC],
            "W": W,
            "b": b,
        }
        for c in range(NCORES)
    ]
    res = run_bass_kernel_spmd(nc, in_maps, list(range(NCORES)))
    last_results = res
    out = np.concatenate(
        [res.results[c]["context"] for c in range(NCORES)], axis=0
    )
    return out.astype(np.float32, copy=False)


# revision 16
# speedup vs baseline: 6.6809x; 6.6809x over previous
"""Trainium2 Bass kernel for the ContextComputer GNN message-passing module.

Computation (per batch row b):
    W1, W2 = W[:D], W[D:]
    u_j    = memory_j * mask_j                       # [N, D]
    a_i    = memory_i @ W1 + bias                    # [N, D]
    c_j    = mask_j * (memory_j @ W2)                # [N, D]
    ctx_i  = sum_{j != i} sigmoid(a_i + c_j) * u_j

Sharding: pure data parallel over batch B across the 8 NeuronCores
(B=8192 -> 1024 rows per core); W/b replicated.

Per-core layout: batch rows on the 128 SBUF partitions, features on the
free axis. v2 design notes:
  - memory is cast-loaded fp32->bf16 straight into SBUF (SWDGE cast DMA,
    no DRAM scratch), and the d-on-partition tiles for the matmuls come
    from ONE SBUF->SBUF xbar transpose per row-tile ([128, 3072] ->
    24 planes of [128d, 128rows]).
  - a'_i = m_i @ W1 + 1*bias accumulates in PSUM (bias via a rank-1 ones
    matmul); c_j = mask_j * (m_j @ W2) applies the mask as a per-partition
    scale in the ScalarE PSUM->SBUF copy.
  - Pairwise stage is diagonal-exclusive: per i, the 5 off-diagonal j's
    are the two contiguous runs [0,i) and (i,N); t/g/q tiles hold the 5
    compressed j-slots (2 DVE adds, 1 ScalarE sigmoid, 2 DVE mults per i).
  - The masked j-sum runs on the Tensor engine: 5 matmuls with
    lhsT=diag(mask_j) accumulate q_ij = g_ij*m_j (SBUF bf16) into PSUM
    fp32 (applying the outer mask_j for free), then one DVE copy writes
    the fp32 output tile. This replaces both the old u_j = mask_j*m_j
    tensor_scalars and the DVE reduction tree, and improves accumulation
    precision (fp32). diag(mask_j) tiles are built by GpSimd
    affine_select from the mask column (GpSimd is otherwise idle).
  - Emission is software-pipelined one row-tile ahead (produce stage of
    tile k+1 before pairwise stage of tile k), and within the pairwise
    stage mult(i) is emitted two adds behind sigmoid(i) so neither the
    DVE nor ACT in-order FIFO ever stalls on the other engine.
"""

import numpy as np

import concourse.bass as bass
import concourse.mybir as mybir
import concourse.tile as tile
from concourse.bass_utils import run_bass_kernel_spmd

B, N, D = 8192, 6, 512
P = 128
DC = D // P  # 4 contraction chunks of 128
NCORES = 8
BLOC = B // NCORES

F32 = mybir.dt.float32
BF16 = mybir.dt.bfloat16

_ADD = mybir.AluOpType.add
_MULT = mybir.AluOpType.mult
_IS_EQUAL = mybir.AluOpType.is_equal
_SIGMOID = mybir.ActivationFunctionType.Sigmoid

_nc_cache = {}


def _split_excess_waits(nc, max_waits=1):
    """The pinned walrus build only supports one sync-wait slot per
    instruction; hoist extra Tile-emitted waits onto standalone
    same-engine EventSemaphore instructions (NX dispatcher-level waits,
    so ordering semantics are preserved)."""
    f = nc.m.functions[0]
    for blk in f.blocks:
        new = []
        for ins in blk.instructions:
            si = getattr(ins, "sync_info", None)
            eng = getattr(ins, "engine", None)
            if si is not None and si.on_wait and len(si.on_wait) > max_waits and eng is not None:
                waits = list(si.on_wait)
                extra, keep = waits[:-max_waits], waits[-max_waits:]
                for k, w in enumerate(extra):
                    new.append(
                        mybir.InstEventSemaphore(
                            name=f"{ins.name}_xw{k}",
                            opcode="EventSemaphore",
                            engine=eng,
                            ins=[],
                            outs=[],
                            sync_info=mybir.SyncInfo(on_wait=[w], on_update=[]),
                        )
                    )
                si.on_wait = keep
            new.append(ins)
        blk.instructions[:] = new


def build(bloc=BLOC, split_waits=True, reps=1):
    """reps > 1 unrolls the whole computation R times inside the NEFF
    (same inputs/outputs each rep) — a timing-only variant that amortizes
    the per-launch client/proxy overhead so steady-state per-execution HW
    time is observable from wall-clock measurements."""
    nbt = bloc // P
    nc = bass.Bass(num_swdge_queues=4)
    mem = nc.declare_dram_parameter("memory", [bloc, N, D], F32, isOutput=False)
    msk = nc.declare_dram_parameter("mask", [bloc, N, 1], F32, isOutput=False)
    w_p = nc.declare_dram_parameter("W", [2 * D, D], F32, isOutput=False)
    b_p = nc.declare_dram_parameter("b", [D], F32, isOutput=False)
    out = nc.declare_dram_parameter("context", [bloc, N, D], F32, isOutput=True)

    with tile.TileContext(nc) as tc:
        with (
            tc.tile_pool(name="const", bufs=1) as constp,
            tc.tile_pool(name="m", bufs=4) as mp,
            tc.tile_pool(name="mt", bufs=4) as mtp,
            tc.tile_pool(name="maskp", bufs=4) as maskp,
            tc.tile_pool(name="up", bufs=3) as upool,
            tc.tile_pool(name="ap", bufs=4) as apool,
            tc.tile_pool(name="cp", bufs=4) as cpool,
            tc.tile_pool(name="pair", bufs=4) as pairp,
            tc.tile_pool(name="outp", bufs=8) as outp,
            tc.tile_pool(name="psum", bufs=2, space="PSUM") as psp,
            tc.tile_pool(name="ctxps", bufs=4, space="PSUM") as ctxps,
        ):
            # ---- constants: W (cast to bf16), bias, ones row, identity ----
            wt = {}
            for h in range(2):  # 0 -> W1, 1 -> W2
                for dc in range(DC):
                    t = constp.tile([P, D], BF16, tag=f"w{h}{dc}")
                    nc.gpsimd.dma_start(
                        out=t[:], in_=w_p[h * D + dc * P : h * D + (dc + 1) * P, :]
                    )
                    wt[h, dc] = t
            bias_t = constp.tile([1, D], BF16, tag="bias")
            nc.gpsimd.dma_start(out=bias_t[:], in_=b_p[None, :])
            ones_t = constp.tile([1, P], BF16, tag="ones")
            nc.vector.memset(ones_t[:], 1.0)
            ident = constp.tile([P, P], BF16, tag="ident")
            nc.vector.memset(ident[:], 1.0)
            nc.gpsimd.affine_select(
                out=ident[:],
                in_=ident[:],
                pattern=[[-1, P]],
                base=0,
                channel_multiplier=1,
                compare_op=_IS_EQUAL,
                fill=0.0,
            )

            state = {}

            def produce(key, bt):
                """loads + transpose + diag-masks + matmuls + PSUM->SBUF copies"""
                bsl = slice(bt * P, (bt + 1) * P)
                m_t = mp.tile([P, N * D], BF16, tag="m")
                nc.gpsimd.dma_start(
                    out=m_t.rearrange("p (n d) -> p n d", n=N), in_=mem[bsl]
                )
                mask_t = maskp.tile([P, N], F32, tag="mask")
                nc.sync.dma_start(out=mask_t[:], in_=msk[bsl, :, 0])
                # one SBUF->SBUF xbar transpose: planes [n*DC+dc] = [128d, 128rows]
                mt_t = mtp.tile([P, N * DC, P], BF16, tag="mt")
                nc.sync.dma_start(out=mt_t[:], in_=m_t[:], transpose=True)
                # diag(mask_j) [P, P] bf16 per j: lhsT for the masked j-sum
                # matmuls (GpSimd affine_select, in_ = mask col broadcast)
                dm_t = upool.tile([P, N, P], BF16, tag="dm")
                for j in range(N):
                    nc.gpsimd.affine_select(
                        out=dm_t[:, j, :],
                        in_=mask_t[:, j : j + 1].broadcast_to([P, P]),
                        pattern=[[-1, P]],
                        base=0,
                        channel_multiplier=1,
                        compare_op=_IS_EQUAL,
                        fill=0.0,
                    )
                a_all = apool.tile([P, N * D], BF16, tag="a")
                c_all = cpool.tile([P, N * D], BF16, tag="c")
                for i in range(N):
                    a_ps = psp.tile([P, D], F32, tag="aps")
                    for dc in range(DC):
                        nc.tensor.matmul(
                            out=a_ps[:],
                            lhsT=mt_t[:, i * DC + dc, :],
                            rhs=wt[0, dc][:],
                            start=(dc == 0),
                            stop=False,
                        )
                    nc.tensor.matmul(
                        out=a_ps[:],
                        lhsT=ones_t[:],
                        rhs=bias_t[:],
                        start=False,
                        stop=True,
                    )
                    nc.scalar.copy(out=a_all[:, i * D : (i + 1) * D], in_=a_ps[:])
                for j in range(N):
                    c_ps = psp.tile([P, D], F32, tag="cps")
                    for dc in range(DC):
                        nc.tensor.matmul(
                            out=c_ps[:],
                            lhsT=mt_t[:, j * DC + dc, :],
                            rhs=wt[1, dc][:],
                            start=(dc == 0),
                            stop=(dc == DC - 1),
                        )
                    # c_j = mask_j * (m_j @ W2): scale in the PSUM->SBUF copy
                    nc.scalar.mul(
                        out=c_all[:, j * D : (j + 1) * D],
                        in_=c_ps[:],
                        mul=mask_t[:, j : j + 1],
                    )
                state[key] = (m_t, dm_t, a_all, c_all)

            def pairwise(key, bt):
                """adds + sigmoid + mult + PE masked j-sum + output DMA.
                Diagonal-exclusive: per i, the 5 off-diagonal j's are the
                two contiguous runs [0, i) and (i, N); t/g/q tiles hold the
                5 compressed j-slots. Emission is skewed so DVE's mult(i)
                (which waits on ACT's sigmoid(i)) is queued two adds later,
                keeping both engine FIFOs bubble-free."""
                bsl = slice(bt * P, (bt + 1) * P)
                m_t, dm_t, a_all, c_all = state.pop(key)
                g_tiles = {}
                s_tiles = {}

                def emit_adds_sig(i):
                    runs = [(0, 0, i), (i, i + 1, N)]  # (slot0, j0, j1)
                    t_i = pairp.tile([P, (N - 1) * D], BF16, tag="t")
                    for slot0, j0, j1 in runs:
                        if j1 == j0:
                            continue
                        nj = j1 - j0
                        a_b = (
                            a_all[:, i * D : (i + 1) * D]
                            .rearrange("p (j f) -> p j f", j=1)
                            .broadcast_to([P, nj, D])
                        )
                        nc.vector.tensor_tensor(
                            out=t_i[
                                :, slot0 * D : (slot0 + nj) * D
                            ].rearrange("p (j f) -> p j f", j=nj),
                            in0=a_b,
                            in1=c_all[:, j0 * D : j1 * D].rearrange(
                                "p (j f) -> p j f", j=nj
                            ),
                            op=_ADD,
                        )
                    g_i = pairp.tile([P, (N - 1) * D], BF16, tag="g")
                    nc.scalar.activation(out=g_i[:], in_=t_i[:], func=_SIGMOID)
                    g_tiles[i] = g_i

                def emit_mult_jsum(i):
                    runs = [(0, 0, i), (i, i + 1, N)]
                    g_i = g_tiles.pop(i)
                    # q = g * m_j (mask applied later via diag-mask matmul)
                    q_i = pairp.tile([P, (N - 1) * D], BF16, tag="q")
                    for slot0, j0, j1 in runs:
                        if j1 == j0:
                            continue
                        nj = j1 - j0
                        nc.vector.tensor_tensor(
                            out=q_i[:, slot0 * D : (slot0 + nj) * D],
                            in0=g_i[:, slot0 * D : (slot0 + nj) * D],
                            in1=m_t[:, j0 * D : j1 * D],
                            op=_MULT,
                        )
                    # ctx_i = sum_k diag(mask_jk) @ q_k  (PSUM fp32 accumulate)
                    js = [j for j in range(N) if j != i]
                    s_ps = ctxps.tile([P, D], F32, tag="s")
                    for k, j in enumerate(js):
                        nc.tensor.matmul(
                            out=s_ps[:],
                            lhsT=dm_t[:, j, :],
                            rhs=q_i[:, k * D : (k + 1) * D],
                            start=(k == 0),
                            stop=(k == len(js) - 1),
                        )
                    s_tiles[i] = s_ps

                SKEW = 2
                for i in range(N):
                    emit_adds_sig(i)
                    if i >= SKEW:
                        emit_mult_jsum(i - SKEW)
                for i in range(N - SKEW, N):
                    emit_mult_jsum(i)
                for i in range(N):
                    s_ps = s_tiles.pop(i)
                    ctx_t = outp.tile([P, D], F32, tag="ctx")
                    nc.vector.tensor_copy(out=ctx_t[:], in_=s_ps[:])
                    nc.scalar.dma_start(out=out[bsl, i, :], in_=ctx_t[:])

            prev = None
            for rep in range(reps):
                for bt in range(nbt):
                    key = (rep, bt)
                    produce(key, bt)
                    if prev is not None:
                        pairwise(prev, prev[1])
                    prev = key
            pairwise(prev, prev[1])
    if split_waits:
        _split_excess_waits(nc)
    return nc


def get_nc(bloc=BLOC):
    if bloc not in _nc_cache:
        _nc_cache[bloc] = build(bloc)
    return _nc_cache[bloc]


last_results = None


def kernel(**inputs):
    global last_results
    memory = np.ascontiguousarray(inputs["memory"], dtype=np.float32)
    mask = np.ascontiguousarray(inputs["mask"], dtype=np.float32)
    W = np.ascontiguousarray(inputs["W"], dtype=np.float32)
    b = np.ascontiguousarray(inputs["b"], dtype=np.float32)

    nc = get_nc()
    in_maps = [
        {
            "memory": memory[c * BLOC : (c + 1) * BLOC],
            "mask": mask[c * BLOC : (c + 1) * BLOC],
            "W": W,
            "b": b,
        }
        for c in range(NCORES)
    ]
    res = run_bass_kernel_spmd(nc, in_maps, list(range(NCORES)))
    last_results = res
    out = np.concatenate(
        [res.results[c]["context"] for c in range(NCORES)], axis=0
    )
    return out.astype(np.float32, copy=False)
